# revision 1
# baseline (speedup 1.0000x reference)
"""MultiHeadLatentAttention TRN2 kernel.

Sharding: 8 cores = 2 (batch) x 4 (head groups of 4 heads).  Each core
computes, for its batch b and its 4 heads: the projections K^T/Q^T (with
RoPE) and V, full attention, and a partial output projection (its 512
rows of Wo's input dim).  Partial outputs are summed on the host (+ bo).

The latent down-projections (Wkv_d, Wq_d) are folded into the per-head
up-projections ON THE HOST (W_fused = W_up @ W_down, bias folded too), so
the device runs five direct E->heads projections per core and never
materializes the latents.  This costs slightly fewer FLOPs than latents
with replicated down-projections and removes a whole dependency layer.

All matmul operands are bf16 (fp32 PSUM accumulation): full PE rate with
fast-weight-load, half the DMA traffic and SBUF footprint, and 2x DVE
throughput for the softmax row-sum accumulation.  Weights load into SBUF
once.  RoPE's rotate_half is a partition-pair swap: rope feature rows are
host-permuted (pairs (i, i+32) adjacent) so DVE stream_shuffle(mask=i^1)
implements the rotation; the sign lives in the host-built sin table.

Attention (per head, per 1024-query block): 16 key-chunk iterations of
QK matmuls -> exp on ACT (the bottleneck engine) -> bf16 row-sum
accumulation on DVE -> PV matmuls trailing by LAG chunks.  Softmax skips
max-subtraction (scores are bounded).  The PV accumulators drain to SBUF
unnormalized as soon as PV finishes (frees PSUM for the next block); the
row-sum/reciprocal/broadcast/normalize tail is software-pipelined 1-2
blocks behind so its small PE/DVE ops never stall the matmul stream
(row sums and the partition broadcast are tiny ones-vector matmuls).
The attention output overwrites the Q tiles (each column range is fully
consumed before the matching att columns are written), saving SBUF.

Phase order: K/V projections for all s-chunks, Q projections for the
first half, then attention blocks with the remaining Q chunks
interleaved; the output projection (C) for each query half runs behind
the corresponding attention blocks and streams [128, 2048] row blocks
out with one 1 MB DMA per block.
"""
import sys

sys.path.insert(0, "/opt/trn_rl_repo")

from contextlib import ExitStack

import numpy as np

H = 16
E = 2048
LAT = E // 4          # 512
D = E // H            # 128
R = D // 2            # 64
B, S = 2, 2048
HPC = H // 4          # 4 heads per core
NCORES = 8
NE = E // 128         # 16 contraction chunks over E
NL = LAT // 128       # 4 contraction chunks over LAT
SW = 512              # s-chunk width for projections
NSC = S // SW         # 4 s-chunks
NKC = S // 128        # 16 key chunks
SCALE = 1.0 / float(np.sqrt(D))

_RT = {}  # cached runtimes


def _mk(nc):
    """Declare DRAM I/O; returns dict of handles."""
    import concourse.mybir as mybir
    F32 = mybir.dt.float32
    BF16 = mybir.dt.bfloat16
    d = {}
    d["xT"] = nc.dram_tensor("xT", [E, S], BF16, kind="ExternalInput")
    for nm in ("wk1f", "wq1f", "wqrf", "wrkT"):
        d[nm] = nc.dram_tensor(nm, [E, HPC * R], BF16, kind="ExternalInput")
    d["wvf"] = nc.dram_tensor("wvf", [E, HPC * D], BF16,
                              kind="ExternalInput")
    d["woT"] = nc.dram_tensor("woT", [HPC * D, E], BF16,
                              kind="ExternalInput")
    for nm in ("bk1f", "bq1f", "bqrf", "brk"):
        d[nm] = nc.dram_tensor(nm, [128, 2], F32, kind="ExternalInput")
    d["bvf"] = nc.dram_tensor("bvf", [1, HPC * D], F32,
                              kind="ExternalInput")
    d["onesd"] = nc.dram_tensor("onesd", [128, 1], BF16,
                                kind="ExternalInput")
    d["onesr"] = nc.dram_tensor("onesr", [1, 128], BF16,
                                kind="ExternalInput")
    d["cosT"] = nc.dram_tensor("cosT", [128, S], BF16,
                               kind="ExternalInput")
    d["sinT"] = nc.dram_tensor("sinT", [128, S], BF16,
                               kind="ExternalInput")
    d["out"] = nc.dram_tensor("out", [S, E], F32, kind="ExternalOutput")
    return d


def _consts(nc, tc, top, d):
    """Persistent tiles: K/Q/V storage, biases, ones, all weights."""
    import concourse.mybir as mybir
    F32 = mybir.dt.float32
    BF16 = mybir.dt.bfloat16

    kq_pool = top.enter_context(tc.tile_pool(name="kq", bufs=1))
    v_pool = top.enter_context(tc.tile_pool(name="vp", bufs=1))
    cpool = top.enter_context(tc.tile_pool(name="cp", bufs=1))
    wpool = top.enter_context(tc.tile_pool(name="wp", bufs=1))

    t = {}
    t["K"] = [kq_pool.tile([128, S], BF16, name=f"Kt{h}") for h in range(HPC)]
    t["Q"] = [kq_pool.tile([128, S], BF16, name=f"Qt{h}") for h in range(HPC)]
    t["V"] = [v_pool.tile([128, HPC * D], BF16, name=f"Vt{i}")
              for i in range(NKC)]

    def ld(name, dram, shape, dt=F32):
        tl = cpool.tile(shape, dt, name=name)
        nc.sync.dma_start(tl[:], dram[:])
        return tl

    t["ones"] = ld("ones_t", d["onesd"], [128, 1], BF16)
    t["onesr"] = ld("onesr_t", d["onesr"], [1, 128], BF16)
    t["bk1f"] = ld("bk1f_t", d["bk1f"], [128, 2])
    t["bq1f"] = ld("bq1f_t", d["bq1f"], [128, 2])
    t["bqrf"] = ld("bqrf_t", d["bqrf"], [128, 2])
    t["brk"] = ld("brk_t", d["brk"], [128, 2])
    bvf_row = ld("bvf_row", d["bvf"], [1, HPC * D])
    bvf_bc = cpool.tile([128, HPC * D], F32, name="bvf_bc")
    nc.gpsimd.partition_broadcast(bvf_bc[:], bvf_row[:])
    t["bvf_bc"] = bvf_bc

    # fused projection weights, loaded once: [128, NE * cols] with the E
    # contraction dim folded as (e p) -> p e
    dnw = {}
    for nm, key, cw in (("k1f", "wk1f", HPC * R), ("q1f", "wq1f", HPC * R),
                        ("qrf", "wqrf", HPC * R), ("rk", "wrkT", HPC * R),
                        ("vf", "wvf", HPC * D)):
        tl = wpool.tile([128, NE * cw], BF16, name=f"wd{nm}")
        nc.sync.dma_start(
            tl[:].rearrange("p (e c) -> p e c", e=NE),
            d[key][:].rearrange("(e p) c -> p e c", p=128))
        dnw[nm] = (tl, cw)
    t["dnw"] = dnw

    # output projection weights
    t["wo"] = [wpool.tile([128, E], BF16, name=f"wo{hc}")
               for hc in range(HPC)]
    for hc in range(HPC):
        nc.sync.dma_start(t["wo"][hc][:], d["woT"][hc * 128:(hc + 1) * 128, :])
    return t


def _pools(nc, tc, st):
    import concourse.mybir as mybir  # noqa: F401
    p = {}
    p["xa"] = st.enter_context(tc.tile_pool(name="xa", bufs=2))
    p["cs"] = st.enter_context(tc.tile_pool(name="cs", bufs=1))
    p["rp"] = st.enter_context(tc.tile_pool(name="rp", bufs=1))
    p["pe"] = st.enter_context(tc.tile_pool(name="pe", bufs=3))
    p["cb"] = st.enter_context(tc.tile_pool(name="cb", bufs=2))
    p["oc"] = st.enter_context(tc.tile_pool(name="oc", bufs=2))
    # PSUM: pac 2x[128,512] (A chains, C groups, row-sum outputs),
    # psS 2x[128,1024] (score tiles), psO 2x[128,512] (PV accumulators)
    p["pac"] = st.enter_context(tc.tile_pool(name="pac", bufs=2,
                                             space="PSUM"))
    p["psS"] = st.enter_context(tc.tile_pool(name="psS", bufs=2,
                                             space="PSUM"))
    p["psO"] = st.enter_context(tc.tile_pool(name="psO", bufs=1,
                                             space="PSUM"))
    return p


def _emit_A(nc, tc, d, t, p, sc_list=None):
    import concourse.mybir as mybir
    from concourse.alu_op_type import AluOpType
    F32 = mybir.dt.float32
    BF16 = mybir.dt.bfloat16
    K_t, Q_t, V_t = t["K"], t["Q"], t["V"]
    swap_mask = [i ^ 1 for i in range(32)]

    if sc_list is None:
        sc_list = [(True, s) for s in range(NSC)] + \
            [(False, s) for s in range(NSC)]
    for kv_pass, sc in sc_list:
        ssl = slice(sc * SW, (sc + 1) * SW)
        xt = p["xa"].tile([128, NE * SW], BF16, name="xt")
        nc.sync.dma_start(
            xt[:].rearrange("p (e s) -> p e s", e=NE),
            d["xT"][:, ssl].rearrange("(e p) s -> p e s", p=128))
        cos_s = p["cs"].tile([128, SW], BF16, name="cos_s")
        nc.sync.dma_start(cos_s[:], d["cosT"][:, ssl])
        sin_s = p["cs"].tile([128, SW], BF16, name="sin_s")
        nc.sync.dma_start(sin_s[:], d["sinT"][:, ssl])

        def proj(wname, m):
            # x @ W chunk: 16-deep contraction over E, [128, SW] out
            wt, cw = t["dnw"][wname]
            ps = p["pac"].tile([128, SW], F32, name="psA", tag="pac")
            for e in range(NE):
                nc.tensor.matmul(
                    ps[:], wt[:, e * cw + m * 128:e * cw + (m + 1) * 128],
                    xt[:, e * SW:(e + 1) * SW],
                    start=(e == 0), stop=(e == NE - 1))
            return ps

        def rope(ps, bias_t, m, dst):
            # ps: [128 rows = 2 heads x 64 rope rows, SW]
            xb = p["rp"].tile([128, SW], BF16, name="xb")
            nc.vector.tensor_scalar_add(xb[:], ps[:], bias_t[:, m:m + 1])
            sh = p["rp"].tile([128, SW], BF16, name="sh")
            nc.vector.stream_shuffle(sh[:], xb[:], swap_mask)
            t1 = p["rp"].tile([128, SW], BF16, name="t1")
            nc.vector.tensor_tensor(t1[:], xb[:], cos_s[:],
                                    op=AluOpType.mult)
            t2 = p["rp"].tile([128, SW], BF16, name="t2")
            nc.vector.tensor_tensor(t2[:], sh[:], sin_s[:],
                                    op=AluOpType.mult)
            nc.vector.tensor_tensor(dst[2 * m][R:D, ssl], t1[0:R, :],
                                    t2[0:R, :], op=AluOpType.add)
            nc.vector.tensor_tensor(dst[2 * m + 1][R:D, ssl], t1[R:D, :],
                                    t2[R:D, :], op=AluOpType.add)

        if kv_pass:
            for m in range(2):  # k1 -> K rows 0..63 (fused through kv_d)
                ps = proj("k1f", m)
                nc.vector.tensor_scalar_add(
                    K_t[2 * m][0:R, ssl], ps[0:R, :],
                    t["bk1f"][0:R, m:m + 1])
                nc.vector.tensor_scalar_add(
                    K_t[2 * m + 1][0:R, ssl], ps[R:D, :],
                    t["bk1f"][R:D, m:m + 1])
            for m in range(2):  # rope-k from x
                ps = proj("rk", m)
                rope(ps, t["brk"], m, K_t)
            for j in range(SW // 128):  # V (s, feat) layout, fused
                wt, cw = t["dnw"]["vf"]
                ps = p["pac"].tile([128, HPC * D], F32, name="psA",
                                   tag="pac")
                for e in range(NE):
                    nc.tensor.matmul(
                        ps[:],
                        xt[:, e * SW + j * 128:e * SW + (j + 1) * 128],
                        wt[:, e * cw:(e + 1) * cw],
                        start=(e == 0), stop=(e == NE - 1))
                nc.vector.tensor_tensor(V_t[sc * (SW // 128) + j][:],
                                        ps[:], t["bvf_bc"][:],
                                        op=AluOpType.add)
        else:
            for m in range(2):  # q1 + rope-q for both head pairs
                ps = proj("q1f", m)
                nc.vector.tensor_scalar_add(
                    Q_t[2 * m][0:R, ssl], ps[0:R, :],
                    t["bq1f"][0:R, m:m + 1])
                nc.vector.tensor_scalar_add(
                    Q_t[2 * m + 1][0:R, ssl], ps[R:D, :],
                    t["bq1f"][R:D, m:m + 1])
                ps = proj("qrf", m)
                rope(ps, t["bqrf"], m, Q_t)


def _emit_B_half(nc, tc, d, t, p, att_t, qp, mode="full",
                 between=None):
    import concourse.mybir as mybir
    from concourse.alu_op_type import AluOpType
    F32 = mybir.dt.float32
    BF16 = mybir.dt.bfloat16
    AF = mybir.ActivationFunctionType
    K_t, Q_t, V_t = t["K"], t["Q"], t["V"]

    LAG = 4  # PV trails QK/exp by LAG k-chunks so PE never waits on ACT

    def stream(h):
        """QK/exp/row-acc/PV for head h; returns a deferred-tail closure.

        The softmax tail (row-sum matmuls, 1/r, broadcast, normalize) is
        emitted one block later so its tiny PE/DVE ops land BEHIND the
        next block's matmul stream in each engine's in-order queue
        instead of stalling it.  The PV accumulators are drained to SBUF
        (unnormalized, into the att tiles) as soon as PV finishes, which
        frees the PSUM slots for the next block.
        """
        qa = slice(qp * 1024, qp * 1024 + 512)
        qb = slice(qp * 1024 + 512, (qp + 1) * 1024)
        oA = p["psO"].tile([128, 512], F32, name="oA")
        oB = p["psO"].tile([128, 512], F32, name="oB")
        acc0 = p["pe"].tile([128, 1024], BF16, name="acc0", bufs=2)
        acc1 = p["pe"].tile([128, 1024], BF16, name="acc1", bufs=2)
        accs = (acc0, acc1)
        pes = {}

        def pv(kk):
            pet = pes.pop(kk)
            nc.tensor.matmul(oA[:], V_t[kk][:, h * D:(h + 1) * D],
                             pet[:, 0:512], start=(kk == 0),
                             stop=(kk == NKC - 1))
            nc.tensor.matmul(oB[:], V_t[kk][:, h * D:(h + 1) * D],
                             pet[:, 512:1024], start=(kk == 0),
                             stop=(kk == NKC - 1))

        for kk in range(NKC):
            ksl = slice(kk * 128, (kk + 1) * 128)
            pp = p["psS"].tile([128, 1024], F32, name="pp")
            nc.tensor.matmul(pp[:, 0:512], K_t[h][:, ksl], Q_t[h][:, qa],
                             start=True, stop=True)
            nc.tensor.matmul(pp[:, 512:1024], K_t[h][:, ksl],
                             Q_t[h][:, qb], start=True, stop=True)
            if mode == "qk":
                continue
            pet = p["pe"].tile([128, 1024], BF16, name="pet", bufs=7)
            nc.scalar.activation(pet[:], pp[:], AF.Exp, scale=SCALE)
            if mode == "qke":
                continue
            if mode != "qkep":
                acc = accs[kk % 2]
                if kk < 2:
                    nc.vector.tensor_copy(acc[:], pet[:])
                else:
                    nc.vector.tensor_tensor(acc[:], pet[:], acc[:],
                                            op=AluOpType.add)
            if mode != "qkea":
                pes[kk] = pet
                if kk >= LAG:
                    pv(kk - LAG)
        if mode != "full":
            return None
        for kk in range(NKC - LAG, NKC):
            pv(kk)
        nc.vector.tensor_copy(att_t[h][:, qa], oA[:])
        nc.vector.tensor_copy(att_t[h][:, qb], oB[:])
        nc.vector.tensor_tensor(acc0[:], acc1[:], acc0[:],
                                op=AluOpType.add)

        def tail1():
            sumA = p["pac"].tile([1, 512], F32, name="sumA", tag="pac")
            nc.tensor.matmul(sumA[:], t["ones"][:], acc0[:, 0:512],
                             start=True, stop=True)
            sumB = p["pac"].tile([1, 512], F32, name="sumB", tag="pac")
            nc.tensor.matmul(sumB[:], t["ones"][:], acc0[:, 512:1024],
                             start=True, stop=True)
            ci = p["cb"].tile([1, 1024], BF16, name="ci")
            with nc.allow_low_precision("softmax denom recip in bf16"):
                nc.vector.reciprocal(ci[:, 0:512], sumA[:])
                nc.vector.reciprocal(ci[:, 512:1024], sumB[:])
            return ci

        def tail2(ci):
            bcA = p["pac"].tile([128, 512], F32, name="bcA", tag="pac")
            nc.tensor.matmul(bcA[:], t["onesr"][:], ci[:, 0:512],
                             start=True, stop=True)
            bcB = p["pac"].tile([128, 512], F32, name="bcB", tag="pac")
            nc.tensor.matmul(bcB[:], t["onesr"][:], ci[:, 512:1024],
                             start=True, stop=True)
            nc.vector.tensor_tensor(att_t[h][:, qa], att_t[h][:, qa],
                                    bcA[:], op=AluOpType.mult)
            nc.vector.tensor_tensor(att_t[h][:, qb], att_t[h][:, qb],
                                    bcB[:], op=AluOpType.mult)

        return tail1, tail2

    # 3-stage software pipeline: block h's row-sum+recip lands behind
    # stream h+1; its broadcast+normalize lands behind stream h+2 -- all
    # tail ops enter each engine's in-order queue with their inputs
    # already long since computed, so nothing stalls the matmul stream.
    from collections import deque
    pend = deque()  # (tail1, tail2, ci)
    for h in range(HPC):
        if between is not None:
            between(h)
        pair = stream(h)
        if pair is None:
            continue
        pend.append(list(pair) + [None])
        if len(pend) >= 2:
            e = pend[-2]
            e[2] = e[0]()  # tail1 of previous block
        if len(pend) >= 3:
            e = pend.popleft()
            if e[2] is None:
                e[2] = e[0]()
            e[1](e[2])  # tail2 of block h-2
    while pend:
        e = pend.popleft()
        if e[2] is None:
            e[2] = e[0]()
        e[1](e[2])


def _emit_C_half(nc, tc, d, t, p, att_t, qp):
    import concourse.mybir as mybir
    F32 = mybir.dt.float32

    for sj in range(qp * 8, (qp + 1) * 8):
        ob = p["oc"].tile([128, E], F32, name="ob")
        for ocn in range(E // 512):
            ps = p["pac"].tile([128, 512], F32, name="psC", tag="pac")
            for hc in range(HPC):
                nc.tensor.matmul(ps[:],
                                 att_t[hc][:, sj * 128:(sj + 1) * 128],
                                 t["wo"][hc][:, ocn * 512:(ocn + 1) * 512],
                                 start=(hc == 0), stop=(hc == HPC - 1))
            nc.scalar.copy(ob[:, ocn * 512:(ocn + 1) * 512], ps[:])
        nc.sync.dma_start(d["out"][sj * 128:(sj + 1) * 128, :], ob[:])


def _emit_body(nc, tc, d, t, p, att_t):
    _emit_A(nc, tc, d, t, p,
            [(True, s) for s in range(NSC)] + [(False, 0), (False, 1)])

    def between(h):
        if h in (1, 2):  # late Q chunks between the first B blocks
            _emit_A(nc, tc, d, t, p, [(False, h + 1)])

    _emit_B_half(nc, tc, d, t, p, att_t, 0, between=between)
    _emit_C_half(nc, tc, d, t, p, att_t, 0)
    _emit_B_half(nc, tc, d, t, p, att_t, 1)
    _emit_C_half(nc, tc, d, t, p, att_t, 1)


def _build_program(loop=None):
    """loop=None: normal kernel. loop=(phase, n): benchmark variant with a
    hardware For_i loop repeating one phase (or the full body) n times."""
    import concourse.bacc as bacc
    import concourse.mybir as mybir
    import concourse.tile as tile

    nc = bacc.Bacc("TRN2", target_bir_lowering=False, debug=False,
                   num_devices=NCORES)
    d = _mk(nc)

    with tile.TileContext(nc) as tc, ExitStack() as top:
        t = _consts(nc, tc, top, d)
        att_t = t["Q"]  # att output overwrites consumed Q columns
        p = _pools(nc, tc, top)
        if loop is None:
            _emit_body(nc, tc, d, t, p, att_t)
        else:
            phase, n = loop

            def _fill(tile_, w):
                nc.sync.dma_start(tile_[:], d["xT"][0:128, 0:w])

            if phase == "A":
                with tc.For_i(0, n, 1):
                    _emit_A(nc, tc, d, t, p)
            elif phase.startswith("B"):
                mode = {"B": "full", "B0": "qk", "B1": "qke",
                        "B2": "qkep", "B3": "qkea", "B4": "notail"}[phase]
                for h in range(HPC):
                    _fill(t["K"][h], S)
                    _fill(t["Q"][h], S)
                for i in range(NKC):
                    _fill(t["V"][i], HPC * D)
                with tc.For_i(0, n, 1):
                    for qp in range(2):
                        _emit_B_half(nc, tc, d, t, p, att_t, qp, mode)
            elif phase == "C":
                for h in range(HPC):
                    _fill(att_t[h], S)
                with tc.For_i(0, n, 1):
                    for qp in range(2):
                        _emit_C_half(nc, tc, d, t, p, att_t, qp)
            elif phase == "full":
                with tc.For_i(0, n, 1):
                    _emit_body(nc, tc, d, t, p, att_t)
            else:
                raise ValueError(phase)

    nc.compile()
    return nc


def _rope_tables():
    inv_freq = 1.0 / (10000.0 ** (np.arange(0, R, 2, dtype=np.float64) / R))
    t = np.arange(S, dtype=np.float64)
    freqs = np.outer(t, inv_freq)                       # (S, R/2)
    emb = np.concatenate([freqs, freqs], axis=-1)       # (S, R)
    cos = np.cos(emb).astype(np.float32)                # (S, R)
    sin = np.sin(emb).astype(np.float32)
    perm = np.array([(j // 2) if j % 2 == 0 else (j // 2) + R // 2
                     for j in range(R)])
    sign = np.array([-1.0 if j % 2 == 0 else 1.0
                     for j in range(R)], dtype=np.float32)
    cos_p = cos[:, perm].T.copy()                       # (R, S)
    sin_p = (sin[:, perm] * sign[None, :]).T.copy()     # (R, S)
    cosT = np.concatenate([cos_p, cos_p], axis=0)       # (128, S)
    sinT = np.concatenate([sin_p, sin_p], axis=0)
    return cosT, sinT, perm


def _bf16():
    import concourse.mybir as mybir
    return mybir.dt.np(mybir.dt.bfloat16)


def _per_core_inputs(inputs, core):
    b, hg = divmod(core, HPC)
    cosT, sinT, perm = _rope_tables()
    hsl64 = np.concatenate([hg * HPC * R + h * R + perm
                            for h in range(HPC)])       # permuted rope rows
    hs64 = slice(hg * HPC * R, (hg + 1) * HPC * R)      # natural 64-rows
    hs128 = slice(hg * HPC * D, (hg + 1) * HPC * D)     # natural 128-rows

    x = np.asarray(inputs["x"], dtype=np.float32)
    f = np.float32
    bf = _bf16()

    def c(a, dt=None):
        return np.ascontiguousarray(a).astype(dt if dt is not None else bf)

    g = {k: np.asarray(v, f) for k, v in inputs.items()}
    # fold the latent down-projections into the per-head up-projections
    wk1 = g["Wk_u"][hs64] @ g["Wkv_d"]            # (256, E)
    bk1 = g["bk_u"][hs64] + g["Wk_u"][hs64] @ g["bkv_d"]
    wv = g["Wv_u"][hs128] @ g["Wkv_d"]            # (512, E)
    bv = g["bv_u"][hs128] + g["Wv_u"][hs128] @ g["bkv_d"]
    wq1 = g["Wq_u"][hs64] @ g["Wq_d"]             # (256, E)
    bq1 = g["bq_u"][hs64] + g["Wq_u"][hs64] @ g["bq_d"]
    wqr = (g["Wrq"] @ g["Wq_d"])[hsl64]           # (256, E), rope-permuted
    bqr = (g["brq"] + g["Wrq"] @ g["bq_d"])[hsl64]

    im = {
        "xT": c(x[b].T),
        "wk1f": c(wk1.T),
        "wq1f": c(wq1.T),
        "wqrf": c(wqr.T),
        "wrkT": c(g["Wrk"][hsl64].T),
        "wvf": c(wv.T),
        "woT": c(g["Wo"].T[hs128]),
        "bk1f": c(bk1.reshape(2, 128).T, f),
        "bq1f": c(bq1.reshape(2, 128).T, f),
        "bqrf": c(bqr.reshape(2, 128).T, f),
        "brk": c(g["brk"][hsl64].reshape(2, 128).T, f),
        "bvf": c(bv.reshape(1, HPC * D), f),
        "onesd": np.ones((128, 1), dtype=bf),
        "onesr": np.ones((1, 128), dtype=bf),
        "cosT": cosT.astype(bf),
        "sinT": sinT.astype(bf),
    }
    return im


def _get_runtime(loop=None, donate=True):
    key = (loop, donate)
    if key in _RT:
        return _RT[key]
    import jax
    import numpy as _np
    from jax.sharding import Mesh, PartitionSpec
    from jax.experimental.shard_map import shard_map

    import concourse.mybir as mybir
    from concourse import bass2jax

    nc = _build_program(loop)
    bass2jax.install_neuronx_cc_hook()

    partition_name = (nc.partition_id_tensor.name
                      if nc.partition_id_tensor else None)
    in_names, out_names, out_avals, zero_shapes = [], [], [], []
    for alloc in nc.m.functions[0].allocations:
        if not isinstance(alloc, mybir.MemoryLocationSet):
            continue
        name = alloc.memorylocations[0].name
        if alloc.kind == "ExternalInput":
            if name != partition_name:
                in_names.append(name)
        elif alloc.kind == "ExternalOutput":
            out_names.append(name)
            np_dt = mybir.dt.np(alloc.dtype)
            out_avals.append(jax.core.ShapedArray(
                tuple(alloc.tensor_shape), np_dt))
            zero_shapes.append((tuple(alloc.tensor_shape), np_dt))

    n_params = len(in_names)
    n_outs = len(out_names)
    all_in_names = list(in_names) + list(out_names)
    if partition_name is not None:
        all_in_names.append(partition_name)

    def _body(*args):
        operands = list(args)
        if partition_name is not None:
            operands.append(bass2jax.partition_id_tensor())
        outs = bass2jax._bass_exec_p.bind(
            *operands,
            out_avals=tuple(out_avals),
            in_names=tuple(all_in_names),
            out_names=tuple(out_names),
            lowering_input_output_aliases=(),
            sim_require_finite=True,
            sim_require_nnan=True,
            nc=nc,
        )
        return tuple(outs)

    devices = jax.devices()[:NCORES]
    mesh = Mesh(_np.asarray(devices), ("core",))
    in_specs = (PartitionSpec("core"),) * (n_params + n_outs)
    out_specs = (PartitionSpec("core"),) * n_outs
    donate_idx = (tuple(range(n_params, n_params + n_outs)) if donate
                  else ())
    sharded = jax.jit(
        shard_map(_body, mesh=mesh, in_specs=in_specs, out_specs=out_specs,
                  check_rep=False),
        donate_argnums=donate_idx, keep_unused=True)

    _RT[key] = dict(sharded=sharded, in_names=in_names, out_names=out_names,
                    zero_shapes=zero_shapes, n_outs=n_outs)
    return _RT[key]


def _run_cores(in_maps):
    rt = _get_runtime()
    import numpy as _np
    concat_in = [
        _np.concatenate([in_maps[c][name] for c in range(NCORES)], axis=0)
        for name in rt["in_names"]
    ]
    concat_zeros = [
        _np.zeros((NCORES * shp[0],) + shp[1:], dt)
        for (shp, dt) in rt["zero_shapes"]
    ]
    out_arrs = rt["sharded"](*concat_in, *concat_zeros)
    res = []
    for c in range(NCORES):
        m = {}
        for i, name in enumerate(rt["out_names"]):
            shp, dt = rt["zero_shapes"][i]
            m[name] = _np.asarray(out_arrs[i]).reshape((NCORES,) + shp)[c]
        res.append(m)
    return res


def kernel(**inputs):
    in_maps = [_per_core_inputs(inputs, c) for c in range(NCORES)]
    res = _run_cores(in_maps)
    bo = np.asarray(inputs["bo"], dtype=np.float32)
    final = np.empty((B, S, E), dtype=np.float32)
    for b in range(B):
        acc = res[HPC * b]["out"].astype(np.float32).copy()
        for g in range(1, HPC):
            acc += res[HPC * b + g]["out"]
        final[b] = acc + bo[None, :]
    return final



# revision 6
# speedup vs baseline: 1.0993x; 1.0993x over previous
"""MultiHeadLatentAttention TRN2 kernel (v2).

Sharding: 8 cores = 2 (batch) x 4 (head groups of 4 heads).  Each core
computes, for its batch b and its 4 heads: the projections K^T/Q^T (with
RoPE) and V, full attention, and a partial output projection (its 512
rows of Wo's input dim).  Partial outputs (bf16) are summed on the host
in fp32 (+ bo).

The latent down-projections (Wkv_d, Wq_d) are folded into the per-head
up-projections ON THE HOST (W_fused = W_up @ W_down, bias folded too).
All matmul operands are bf16 (fp32 PSUM accumulation).

v2 schedule, engine-balanced around the PE array (the global bottleneck)
with the ACT exp stream (B's bottleneck) hidden under interleaved PE
work, and all PSUM drains moved off ACT/DVE onto the idle Pool engine:

  A: 4 merged passes (one x-chunk load each; kv-side all 4, q-side for
     s-chunks 0,1 inline).  cos/sin tables resident.
  B half 0: 4 head streams; the q-side projections for s-chunks 2,3 are
     interleaved between streams (their xt tiles are still resident) so
     the PE has exp-independent work while ACT catches up.
  B half 1: 4 head streams with 6 of C-half-0's output blocks
     interleaved between streams; remaining C0 blocks follow.
  C half 1 at the end; in the For_i steady state it overlaps the next
     iteration's A-phase DMA/DVE leads.

  B stream: QK (2 MM) -> exp on ACT -> bf16 row-accum on DVE -> PV
  trailing by LAG chunks.  PV accumulators drain PSUM->SBUF (bf16 att,
  overwriting consumed Q columns) on Pool.  Row sums: ones-matmul (PE),
  reciprocal on DVE, partition-broadcast on Pool, normalize mult on DVE,
  software-pipelined 1-2 streams behind.  C blocks: 16 accumulating MMs
  per [128 s, 2048] row block, drained PSUM->bf16 SBUF on Pool, one
  0.5 MB DMA out per block.
"""
import sys

sys.path.insert(0, "/opt/trn_rl_repo")

from contextlib import ExitStack

import numpy as np

H = 16
E = 2048
LAT = E // 4          # 512
D = E // H            # 128
R = D // 2            # 64
B, S = 2, 2048
HPC = H // 4          # 4 heads per core
NCORES = 8
NE = E // 128         # 16 contraction chunks over E
NL = LAT // 128       # 4 contraction chunks over LAT
SW = 512              # s-chunk width for projections
NSC = S // SW         # 4 s-chunks
NKC = S // 128        # 16 key chunks
SCALE = 1.0 / float(np.sqrt(D))
LAG = 5               # PV trails QK/exp by LAG k-chunks

_RT = {}  # cached runtimes


def _mk(nc):
    """Declare DRAM I/O; returns dict of handles."""
    import concourse.mybir as mybir
    F32 = mybir.dt.float32
    BF16 = mybir.dt.bfloat16
    d = {}
    d["xT"] = nc.dram_tensor("xT", [E, S], BF16, kind="ExternalInput")
    for nm in ("wk1f", "wq1f", "wqrf", "wrkT"):
        d[nm] = nc.dram_tensor(nm, [E, HPC * R], BF16, kind="ExternalInput")
    d["wvf"] = nc.dram_tensor("wvf", [E, HPC * D], BF16,
                              kind="ExternalInput")
    d["woT"] = nc.dram_tensor("woT", [HPC * D, E], BF16,
                              kind="ExternalInput")
    for nm in ("bk1f", "bq1f", "bqrf", "brk"):
        d[nm] = nc.dram_tensor(nm, [128, 2], F32, kind="ExternalInput")
    d["bvf"] = nc.dram_tensor("bvf", [1, HPC * D], F32,
                              kind="ExternalInput")
    d["onesd"] = nc.dram_tensor("onesd", [128, 1], BF16,
                                kind="ExternalInput")
    d["cosT"] = nc.dram_tensor("cosT", [128, S], BF16,
                               kind="ExternalInput")
    d["sinT"] = nc.dram_tensor("sinT", [128, S], BF16,
                               kind="ExternalInput")
    d["out"] = nc.dram_tensor("out", [S, E], BF16, kind="ExternalOutput")
    return d


def _consts(nc, tc, top, d):
    """Persistent tiles: K/Q/V storage, biases, ones, weights, cos/sin."""
    import concourse.mybir as mybir
    F32 = mybir.dt.float32
    BF16 = mybir.dt.bfloat16

    kq_pool = top.enter_context(tc.tile_pool(name="kq", bufs=1))
    v_pool = top.enter_context(tc.tile_pool(name="vp", bufs=1))
    cpool = top.enter_context(tc.tile_pool(name="cp", bufs=1))
    wpool = top.enter_context(tc.tile_pool(name="wp", bufs=1))

    t = {}
    t["K"] = [kq_pool.tile([128, S], BF16, name=f"Kt{h}") for h in range(HPC)]
    t["Q"] = [kq_pool.tile([128, S], BF16, name=f"Qt{h}") for h in range(HPC)]
    t["V"] = [v_pool.tile([128, HPC * D], BF16, name=f"Vt{i}")
              for i in range(NKC)]

    def ld(name, dram, shape, dt=F32):
        tl = cpool.tile(shape, dt, name=name)
        nc.sync.dma_start(tl[:], dram[:])
        return tl

    t["ones"] = ld("ones_t", d["onesd"], [128, 1], BF16)
    t["bk1f"] = ld("bk1f_t", d["bk1f"], [128, 2])
    t["bq1f"] = ld("bq1f_t", d["bq1f"], [128, 2])
    t["bqrf"] = ld("bqrf_t", d["bqrf"], [128, 2])
    t["brk"] = ld("brk_t", d["brk"], [128, 2])
    t["cos"] = ld("cos_t", d["cosT"], [128, S], BF16)
    t["sin"] = ld("sin_t", d["sinT"], [128, S], BF16)
    bvf_row = ld("bvf_row", d["bvf"], [1, HPC * D])
    bvf_bc = cpool.tile([128, HPC * D], F32, name="bvf_bc")
    nc.gpsimd.partition_broadcast(bvf_bc[:], bvf_row[:])
    t["bvf_bc"] = bvf_bc

    # fused projection weights, loaded once: [128, NE * cols] with the E
    # contraction dim folded as (e p) -> p e
    dnw = {}
    for nm, key, cw in (("k1f", "wk1f", HPC * R), ("q1f", "wq1f", HPC * R),
                        ("qrf", "wqrf", HPC * R), ("rk", "wrkT", HPC * R),
                        ("vf", "wvf", HPC * D)):
        tl = wpool.tile([128, NE * cw], BF16, name=f"wd{nm}")
        nc.sync.dma_start(
            tl[:].rearrange("p (e c) -> p e c", e=NE),
            d[key][:].rearrange("(e p) c -> p e c", p=128))
        dnw[nm] = (tl, cw)
    t["dnw"] = dnw

    # output projection weights
    t["wo"] = [wpool.tile([128, E], BF16, name=f"wo{hc}")
               for hc in range(HPC)]
    for hc in range(HPC):
        nc.sync.dma_start(t["wo"][hc][:], d["woT"][hc * 128:(hc + 1) * 128, :])
    return t


def _pools(nc, tc, st):
    import concourse.mybir as mybir  # noqa: F401
    p = {}
    p["xa"] = st.enter_context(tc.tile_pool(name="xa", bufs=2))
    p["rp"] = st.enter_context(tc.tile_pool(name="rp", bufs=1))
    p["pe"] = st.enter_context(tc.tile_pool(name="pe", bufs=3))
    p["cb"] = st.enter_context(tc.tile_pool(name="cb", bufs=2))
    p["oc"] = st.enter_context(tc.tile_pool(name="oc", bufs=2))
    # PSUM: pac 2x[128,512] (A chains, C groups, row-sum outputs),
    # psS 2x[128,1024] (score tiles), psO 2x[128,512] (PV accumulators)
    p["pac"] = st.enter_context(tc.tile_pool(name="pac", bufs=2,
                                             space="PSUM"))
    p["psS"] = st.enter_context(tc.tile_pool(name="psS", bufs=2,
                                             space="PSUM"))
    p["psO"] = st.enter_context(tc.tile_pool(name="psO", bufs=1,
                                             space="PSUM"))
    return p


def _proj(nc, t, p, wname, m, xt):
    """x @ W chunk: 16-deep contraction over E, [128, SW] psum out."""
    import concourse.mybir as mybir
    F32 = mybir.dt.float32
    wt, cw = t["dnw"][wname]
    ps = p["pac"].tile([128, SW], F32, name="psA", tag="pac")
    for e in range(NE):
        nc.tensor.matmul(
            ps[:], wt[:, e * cw + m * 128:e * cw + (m + 1) * 128],
            xt[:, e * SW:(e + 1) * SW],
            start=(e == 0), stop=(e == NE - 1))
    return ps


def _rope(nc, t, p, ps, bias_t, m, dst, ssl):
    """RoPE rows: ps [128 = 2 heads x 64 rope rows, SW] -> dst rows R:D."""
    import concourse.mybir as mybir
    from concourse.alu_op_type import AluOpType
    BF16 = mybir.dt.bfloat16
    swap_mask = [i ^ 1 for i in range(32)]
    cos_s, sin_s = t["cos"][:, ssl], t["sin"][:, ssl]
    xb = p["rp"].tile([128, SW], BF16, name="xb")
    nc.vector.tensor_scalar_add(xb[:], ps[:], bias_t[:, m:m + 1])
    sh = p["rp"].tile([128, SW], BF16, name="sh")
    nc.vector.stream_shuffle(sh[:], xb[:], swap_mask)
    t1 = p["rp"].tile([128, SW], BF16, name="t1")
    nc.vector.tensor_tensor(t1[:], xb[:], cos_s, op=AluOpType.mult)
    t2 = p["rp"].tile([128, SW], BF16, name="t2")
    nc.vector.tensor_tensor(t2[:], sh[:], sin_s, op=AluOpType.mult)
    nc.vector.tensor_tensor(dst[2 * m][R:D, ssl], t1[0:R, :],
                            t2[0:R, :], op=AluOpType.add)
    nc.vector.tensor_tensor(dst[2 * m + 1][R:D, ssl], t1[R:D, :],
                            t2[R:D, :], op=AluOpType.add)


def _emit_q_side(nc, t, p, sc, xt):
    """q1 + rope-q projections for s-chunk sc from resident xt."""
    ssl = slice(sc * SW, (sc + 1) * SW)
    Q_t = t["Q"]
    for m in range(2):
        ps = _proj(nc, t, p, "q1f", m, xt)
        nc.vector.tensor_scalar_add(
            Q_t[2 * m][0:R, ssl], ps[0:R, :], t["bq1f"][0:R, m:m + 1])
        nc.vector.tensor_scalar_add(
            Q_t[2 * m + 1][0:R, ssl], ps[R:D, :], t["bq1f"][R:D, m:m + 1])
        ps = _proj(nc, t, p, "qrf", m, xt)
        _rope(nc, t, p, ps, t["bqrf"], m, Q_t, ssl)


def _emit_A_pass(nc, tc, d, t, p, sc, do_q):
    """One merged pass for s-chunk sc: xt load, kv-side (+ q-side if
    do_q).  Returns the xt tile (resident until its q-side runs)."""
    import concourse.mybir as mybir
    from concourse.alu_op_type import AluOpType
    F32 = mybir.dt.float32
    BF16 = mybir.dt.bfloat16
    K_t, V_t = t["K"], t["V"]
    ssl = slice(sc * SW, (sc + 1) * SW)

    xt = p["xa"].tile([128, NE * SW], BF16, name="xt")
    nc.sync.dma_start(
        xt[:].rearrange("p (e s) -> p e s", e=NE),
        d["xT"][:, ssl].rearrange("(e p) s -> p e s", p=128))

    for m in range(2):  # k1 -> K rows 0..63 (fused through kv_d)
        ps = _proj(nc, t, p, "k1f", m, xt)
        nc.vector.tensor_scalar_add(
            K_t[2 * m][0:R, ssl], ps[0:R, :], t["bk1f"][0:R, m:m + 1])
        nc.vector.tensor_scalar_add(
            K_t[2 * m + 1][0:R, ssl], ps[R:D, :], t["bk1f"][R:D, m:m + 1])
    for m in range(2):  # rope-k from x
        ps = _proj(nc, t, p, "rk", m, xt)
        _rope(nc, t, p, ps, t["brk"], m, K_t, ssl)
    for j in range(SW // 128):  # V (s, feat) layout, fused
        wt, cw = t["dnw"]["vf"]
        ps = p["pac"].tile([128, HPC * D], F32, name="psA", tag="pac")
        for e in range(NE):
            nc.tensor.matmul(
                ps[:], xt[:, e * SW + j * 128:e * SW + (j + 1) * 128],
                wt[:, e * cw:(e + 1) * cw],
                start=(e == 0), stop=(e == NE - 1))
        nc.vector.tensor_tensor(V_t[sc * (SW // 128) + j][:], ps[:],
                                t["bvf_bc"][:], op=AluOpType.add)
    if do_q:
        _emit_q_side(nc, t, p, sc, xt)
    return xt


def _emit_B_half(nc, tc, d, t, p, att_t, qp, mode="full", between=None):
    import concourse.mybir as mybir
    from concourse.alu_op_type import AluOpType
    F32 = mybir.dt.float32
    BF16 = mybir.dt.bfloat16
    AF = mybir.ActivationFunctionType
    K_t, Q_t, V_t = t["K"], t["Q"], t["V"]

    def stream(h):
        """QK/exp/row-acc/PV for head h; returns deferred-tail closures.

        The softmax tail (row-sum matmuls, 1/r, broadcast, normalize) is
        deferred 1-2 streams so its small ops land BEHIND the next
        stream's work in each engine's in-order queue.  PV accumulators
        drain to SBUF (unnormalized bf16 att tiles) on Pool as soon as
        PV finishes, freeing the PSUM slots.
        """
        qa = slice(qp * 1024, qp * 1024 + 512)
        qb = slice(qp * 1024 + 512, (qp + 1) * 1024)
        oA = p["psO"].tile([128, 512], F32, name="oA")
        oB = p["psO"].tile([128, 512], F32, name="oB")
        acc0 = p["pe"].tile([128, 1024], BF16, name="acc0", bufs=2)
        acc1 = p["pe"].tile([128, 1024], BF16, name="acc1", bufs=2)
        accs = (acc0, acc1)
        pes = {}

        def pv(kk):
            pet = pes.pop(kk)
            nc.tensor.matmul(oA[:], V_t[kk][:, h * D:(h + 1) * D],
                             pet[:, 0:512], start=(kk == 0),
                             stop=(kk == NKC - 1))
            nc.tensor.matmul(oB[:], V_t[kk][:, h * D:(h + 1) * D],
                             pet[:, 512:1024], start=(kk == 0),
                             stop=(kk == NKC - 1))

        for kk in range(NKC):
            ksl = slice(kk * 128, (kk + 1) * 128)
            pp = p["psS"].tile([128, 1024], F32, name="pp")
            nc.tensor.matmul(pp[:, 0:512], K_t[h][:, ksl], Q_t[h][:, qa],
                             start=True, stop=True)
            nc.tensor.matmul(pp[:, 512:1024], K_t[h][:, ksl],
                             Q_t[h][:, qb], start=True, stop=True)
            if mode == "qk":
                continue
            pet = p["pe"].tile([128, 1024], BF16, name="pet", bufs=8)
            nc.scalar.activation(pet[:], pp[:], AF.Exp, scale=SCALE)
            if mode == "qke":
                continue
            if mode != "qkep":
                acc = accs[kk % 2]
                if kk < 2:
                    nc.vector.tensor_copy(acc[:], pet[:])
                else:
                    nc.vector.tensor_tensor(acc[:], pet[:], acc[:],
                                            op=AluOpType.add)
            if mode != "qkea":
                pes[kk] = pet
                if kk >= LAG:
                    pv(kk - LAG)
        if mode != "full":
            return None
        for kk in range(NKC - LAG, NKC):
            pv(kk)
        # drain PV accumulators (f32 PSUM -> bf16 att SBUF) on DVE
        nc.vector.tensor_copy(att_t[h][:, qa], oA[:])
        nc.vector.tensor_copy(att_t[h][:, qb], oB[:])
        nc.vector.tensor_tensor(acc0[:], acc1[:], acc0[:],
                                op=AluOpType.add)

        def tail1():
            sumA = p["pac"].tile([1, 512], F32, name="sumA", tag="pac")
            nc.tensor.matmul(sumA[:], t["ones"][:], acc0[:, 0:512],
                             start=True, stop=True)
            sumB = p["pac"].tile([1, 512], F32, name="sumB", tag="pac")
            nc.tensor.matmul(sumB[:], t["ones"][:], acc0[:, 512:1024],
                             start=True, stop=True)
            ci = p["cb"].tile([1, 1024], BF16, name="ci")
            with nc.allow_low_precision("softmax denom recip in bf16"):
                nc.vector.reciprocal(ci[:, 0:512], sumA[:])
                nc.vector.reciprocal(ci[:, 512:1024], sumB[:])
            return ci

        def tail2(ci):
            bcT = p["cb"].tile([128, 1024], BF16, name="bcT")
            nc.gpsimd.partition_broadcast(bcT[:], ci[:])
            nc.vector.tensor_tensor(att_t[h][:, qa], att_t[h][:, qa],
                                    bcT[:, 0:512], op=AluOpType.mult)
            nc.vector.tensor_tensor(att_t[h][:, qb], att_t[h][:, qb],
                                    bcT[:, 512:1024], op=AluOpType.mult)

        return tail1, tail2

    # 3-stage software pipeline: stream h's row-sum+recip lands behind
    # stream h+1; its broadcast+normalize behind stream h+2.
    from collections import deque
    pend = deque()  # [tail1, tail2, ci]
    for h in range(HPC):
        if between is not None:
            between(h)
        pair = stream(h)
        if pair is None:
            continue
        pend.append(list(pair) + [None])
        if len(pend) >= 2:
            e = pend[-2]
            e[2] = e[0]()
        if len(pend) >= 3:
            e = pend.popleft()
            if e[2] is None:
                e[2] = e[0]()
            e[1](e[2])
    while pend:
        e = pend.popleft()
        if e[2] is None:
            e[2] = e[0]()
        e[1](e[2])


def _emit_C_block(nc, d, t, p, att_t, sj):
    """One [128 s-rows, 2048] output block: 16 accumulating MMs, PSUM
    drained to bf16 on ACT (idle outside the exp stream), one DMA out."""
    import concourse.mybir as mybir
    F32 = mybir.dt.float32
    BF16 = mybir.dt.bfloat16
    ob = p["oc"].tile([128, E], BF16, name="ob")
    for ocn in range(E // 512):
        ps = p["pac"].tile([128, 512], F32, name="psC", tag="pac")
        for hc in range(HPC):
            nc.tensor.matmul(ps[:],
                             att_t[hc][:, sj * 128:(sj + 1) * 128],
                             t["wo"][hc][:, ocn * 512:(ocn + 1) * 512],
                             start=(hc == 0), stop=(hc == HPC - 1))
        nc.scalar.copy(ob[:, ocn * 512:(ocn + 1) * 512], ps[:])
    nc.sync.dma_start(d["out"][sj * 128:(sj + 1) * 128, :], ob[:])


def _emit_C_half(nc, tc, d, t, p, att_t, qp, skip=()):
    for sj in range(qp * 8, (qp + 1) * 8):
        if sj in skip:
            continue
        _emit_C_block(nc, d, t, p, att_t, sj)


def _emit_body(nc, tc, d, t, p, att_t):
    # A: merged passes; q-side inline for s-chunks 0,1 only
    xts = {}
    for sc in range(NSC):
        xts[sc] = _emit_A_pass(nc, tc, d, t, p, sc, do_q=(sc < 2))

    def between0(h):
        # late q-side projections keep the PE fed while ACT runs exps
        if h == 1:
            _emit_q_side(nc, t, p, 2, xts[2])
        elif h == 2:
            _emit_q_side(nc, t, p, 3, xts[3])

    _emit_B_half(nc, tc, d, t, p, att_t, 0, between=between0)

    def between1(h):
        # interleave C half-0 blocks between B half-1 streams
        if h >= 1:
            _emit_C_block(nc, d, t, p, att_t, 2 * (h - 1))
            _emit_C_block(nc, d, t, p, att_t, 2 * (h - 1) + 1)

    _emit_B_half(nc, tc, d, t, p, att_t, 1, between=between1)
    _emit_C_half(nc, tc, d, t, p, att_t, 0, skip=range(6))
    _emit_C_half(nc, tc, d, t, p, att_t, 1)


def _build_program(loop=None):
    """loop=None: normal kernel. loop=(phase, n): benchmark variant with a
    hardware For_i loop repeating one phase (or the full body) n times."""
    import concourse.bacc as bacc
    import concourse.mybir as mybir
    import concourse.tile as tile

    nc = bacc.Bacc("TRN2", target_bir_lowering=False, debug=False,
                   num_devices=NCORES)
    d = _mk(nc)

    with tile.TileContext(nc) as tc, ExitStack() as top:
        t = _consts(nc, tc, top, d)
        att_t = t["Q"]  # att output overwrites consumed Q columns
        p = _pools(nc, tc, top)
        if loop is None:
            _emit_body(nc, tc, d, t, p, att_t)
        else:
            phase, n = loop

            def _loopctx():
                # n == 1: no hardware loop (lets TimelineSim run phases)
                from contextlib import nullcontext
                return tc.For_i(0, n, 1) if n > 1 else nullcontext()

            def _fill(tile_, w):
                nc.sync.dma_start(tile_[:], d["xT"][0:128, 0:w])

            if phase == "A":
                with _loopctx():
                    for sc in range(NSC):
                        _emit_A_pass(nc, tc, d, t, p, sc, do_q=True)
            elif phase.startswith("B"):
                mode = {"B": "full", "B0": "qk", "B1": "qke",
                        "B2": "qkep", "B3": "qkea"}[phase]
                for h in range(HPC):
                    _fill(t["K"][h], S)
                    _fill(t["Q"][h], S)
                for i in range(NKC):
                    _fill(t["V"][i], HPC * D)
                with _loopctx():
                    for qp in range(2):
                        _emit_B_half(nc, tc, d, t, p, att_t, qp, mode)
            elif phase == "C":
                for h in range(HPC):
                    _fill(att_t[h], S)
                with _loopctx():
                    for qp in range(2):
                        _emit_C_half(nc, tc, d, t, p, att_t, qp)
            elif phase == "full":
                with _loopctx():
                    _emit_body(nc, tc, d, t, p, att_t)
            else:
                raise ValueError(phase)

    nc.compile()
    return nc


def _rope_tables():
    inv_freq = 1.0 / (10000.0 ** (np.arange(0, R, 2, dtype=np.float64) / R))
    t = np.arange(S, dtype=np.float64)
    freqs = np.outer(t, inv_freq)                       # (S, R/2)
    emb = np.concatenate([freqs, freqs], axis=-1)       # (S, R)
    cos = np.cos(emb).astype(np.float32)                # (S, R)
    sin = np.sin(emb).astype(np.float32)
    perm = np.array([(j // 2) if j % 2 == 0 else (j // 2) + R // 2
                     for j in range(R)])
    sign = np.array([-1.0 if j % 2 == 0 else 1.0
                     for j in range(R)], dtype=np.float32)
    cos_p = cos[:, perm].T.copy()                       # (R, S)
    sin_p = (sin[:, perm] * sign[None, :]).T.copy()     # (R, S)
    cosT = np.concatenate([cos_p, cos_p], axis=0)       # (128, S)
    sinT = np.concatenate([sin_p, sin_p], axis=0)
    return cosT, sinT, perm


def _bf16():
    import concourse.mybir as mybir
    return mybir.dt.np(mybir.dt.bfloat16)


def _per_core_inputs(inputs, core):
    b, hg = divmod(core, HPC)
    cosT, sinT, perm = _rope_tables()
    hsl64 = np.concatenate([hg * HPC * R + h * R + perm
                            for h in range(HPC)])       # permuted rope rows
    hs64 = slice(hg * HPC * R, (hg + 1) * HPC * R)      # natural 64-rows
    hs128 = slice(hg * HPC * D, (hg + 1) * HPC * D)     # natural 128-rows

    x = np.asarray(inputs["x"], dtype=np.float32)
    f = np.float32
    bf = _bf16()

    def c(a, dt=None):
        return np.ascontiguousarray(a).astype(dt if dt is not None else bf)

    g = {k: np.asarray(v, f) for k, v in inputs.items()}
    # fold the latent down-projections into the per-head up-projections
    wk1 = g["Wk_u"][hs64] @ g["Wkv_d"]            # (256, E)
    bk1 = g["bk_u"][hs64] + g["Wk_u"][hs64] @ g["bkv_d"]
    wv = g["Wv_u"][hs128] @ g["Wkv_d"]            # (512, E)
    bv = g["bv_u"][hs128] + g["Wv_u"][hs128] @ g["bkv_d"]
    wq1 = g["Wq_u"][hs64] @ g["Wq_d"]             # (256, E)
    bq1 = g["bq_u"][hs64] + g["Wq_u"][hs64] @ g["bq_d"]
    wqr = (g["Wrq"] @ g["Wq_d"])[hsl64]           # (256, E), rope-permuted
    bqr = (g["brq"] + g["Wrq"] @ g["bq_d"])[hsl64]

    im = {
        "xT": c(x[b].T),
        "wk1f": c(wk1.T),
        "wq1f": c(wq1.T),
        "wqrf": c(wqr.T),
        "wrkT": c(g["Wrk"][hsl64].T),
        "wvf": c(wv.T),
        "woT": c(g["Wo"].T[hs128]),
        "bk1f": c(bk1.reshape(2, 128).T, f),
        "bq1f": c(bq1.reshape(2, 128).T, f),
        "bqrf": c(bqr.reshape(2, 128).T, f),
        "brk": c(g["brk"][hsl64].reshape(2, 128).T, f),
        "bvf": c(bv.reshape(1, HPC * D), f),
        "onesd": np.ones((128, 1), dtype=bf),
        "cosT": cosT.astype(bf),
        "sinT": sinT.astype(bf),
    }
    return im


def _get_runtime(loop=None, donate=True):
    key = (loop, donate)
    if key in _RT:
        return _RT[key]
    import jax
    import numpy as _np
    from jax.sharding import Mesh, PartitionSpec
    from jax.experimental.shard_map import shard_map

    import concourse.mybir as mybir
    from concourse import bass2jax

    nc = _build_program(loop)
    bass2jax.install_neuronx_cc_hook()

    partition_name = (nc.partition_id_tensor.name
                      if nc.partition_id_tensor else None)
    in_names, out_names, out_avals, zero_shapes = [], [], [], []
    for alloc in nc.m.functions[0].allocations:
        if not isinstance(alloc, mybir.MemoryLocationSet):
            continue
        name = alloc.memorylocations[0].name
        if alloc.kind == "ExternalInput":
            if name != partition_name:
                in_names.append(name)
        elif alloc.kind == "ExternalOutput":
            out_names.append(name)
            np_dt = mybir.dt.np(alloc.dtype)
            out_avals.append(jax.core.ShapedArray(
                tuple(alloc.tensor_shape), np_dt))
            zero_shapes.append((tuple(alloc.tensor_shape), np_dt))

    n_params = len(in_names)
    n_outs = len(out_names)
    all_in_names = list(in_names) + list(out_names)
    if partition_name is not None:
        all_in_names.append(partition_name)

    def _body(*args):
        operands = list(args)
        if partition_name is not None:
            operands.append(bass2jax.partition_id_tensor())
        outs = bass2jax._bass_exec_p.bind(
            *operands,
            out_avals=tuple(out_avals),
            in_names=tuple(all_in_names),
            out_names=tuple(out_names),
            lowering_input_output_aliases=(),
            sim_require_finite=True,
            sim_require_nnan=True,
            nc=nc,
        )
        return tuple(outs)

    devices = jax.devices()[:NCORES]
    mesh = Mesh(_np.asarray(devices), ("core",))
    in_specs = (PartitionSpec("core"),) * (n_params + n_outs)
    out_specs = (PartitionSpec("core"),) * n_outs
    donate_idx = (tuple(range(n_params, n_params + n_outs)) if donate
                  else ())
    sharded = jax.jit(
        shard_map(_body, mesh=mesh, in_specs=in_specs, out_specs=out_specs,
                  check_rep=False),
        donate_argnums=donate_idx, keep_unused=True)

    _RT[key] = dict(sharded=sharded, in_names=in_names, out_names=out_names,
                    zero_shapes=zero_shapes, n_outs=n_outs)
    return _RT[key]


def _run_cores(in_maps):
    rt = _get_runtime()
    import numpy as _np
    concat_in = [
        _np.concatenate([in_maps[c][name] for c in range(NCORES)], axis=0)
        for name in rt["in_names"]
    ]
    concat_zeros = [
        _np.zeros((NCORES * shp[0],) + shp[1:], dt)
        for (shp, dt) in rt["zero_shapes"]
    ]
    out_arrs = rt["sharded"](*concat_in, *concat_zeros)
    res = []
    for c in range(NCORES):
        m = {}
        for i, name in enumerate(rt["out_names"]):
            shp, dt = rt["zero_shapes"][i]
            m[name] = _np.asarray(out_arrs[i]).reshape((NCORES,) + shp)[c]
        res.append(m)
    return res


def kernel(**inputs):
    in_maps = [_per_core_inputs(inputs, c) for c in range(NCORES)]
    res = _run_cores(in_maps)
    bo = np.asarray(inputs["bo"], dtype=np.float32)
    final = np.empty((B, S, E), dtype=np.float32)
    for b in range(B):
        acc = res[HPC * b]["out"].astype(np.float32)
        for g in range(1, HPC):
            acc = acc + res[HPC * b + g]["out"].astype(np.float32)
        final[b] = acc + bo[None, :]
    return final
